# revision 1
# baseline (speedup 1.0000x reference)
"""BiGCN (nn_BiGCN_52716428591487) Trainium2 kernel.

Math: the model's output is log_softmax(cat(l2_bu[root], l2_td[root]) @ W_lin + b).
Only the layer-2 GCN values AT THE ROOT NODES matter, and GCNConv is linear in
its input features, so the whole network collapses to:

  agg1_d[v]  = sum_{e -> v} coef_d(e) * x[nbr(e)] + dinv_d[v]^2 * x[v]   (v in S)
  l1_d[v]    = agg1_d[v] @ W_d1 + b_d1
  cb/ct[v]   = relu([x[root(g(v))], l1_bu/td[v]])
  out2[g]    = sum_{s in S_g} Pr[s, g] * [relu(root), relu(l1_bu), relu(l1_td)][s]
  pb/pt[g]   = relu(out2_{R,bu/td}[g] @ W_2 + b_2)
  out[g]     = log_softmax([pb, pt][g] @ W_lin + b_lin)

where S = {sources of root-incident edges} + {roots} (~1.7k of 50k nodes) and
Pr is the (structure-only) layer-2 aggregation matrix.

Host does index-only preprocessing (degrees, edge selection, gather tables,
Pr); the device does every arithmetic op that touches x: the per-edge
coefficient scaling + aggregation (as one-hot x matmul on the PE), all four
GCN feature transforms, biases, relus, the linear head and log_softmax.

Sharding: graph-data parallel over 8 cores (graphs 0..12 -> core 0, ...).
Each core computes its graphs' rows of the output; the host concatenates.
"""

import numpy as np

P = 128
NCORES = 8


def _roundup(a, m):
    return -(-int(a) // m) * m


# ----------------------------------------------------------------------------
# Host preprocessing: index-only work + gather tables
# ----------------------------------------------------------------------------

def _preprocess(x, edge_index, batch, num_graphs):
    x = np.ascontiguousarray(np.asarray(x), dtype=np.float32)
    ei = np.asarray(edge_index)
    batch = np.asarray(batch).astype(np.int64)
    G = int(np.asarray(num_graphs))
    N, F = x.shape
    src = ei[0].astype(np.int64)
    dst = ei[1].astype(np.int64)

    assert np.all(np.diff(batch) >= 0), "batch must be sorted (contiguous graphs)"
    roots = np.searchsorted(batch, np.arange(G, dtype=np.int64))  # segment_min

    deg_td = 1.0 + np.bincount(dst, minlength=N).astype(np.float64)
    deg_bu = 1.0 + np.bincount(src, minlength=N).astype(np.float64)
    dinv_td = (1.0 / np.sqrt(deg_td)).astype(np.float32)
    dinv_bu = (1.0 / np.sqrt(deg_bu)).astype(np.float32)

    G_cap = max(-(-G // NCORES), 1)

    # S: sources of root-incident edges + roots
    is_root = np.zeros(N, bool)
    is_root[roots] = True
    rmask = is_root[dst]
    r_src, r_dst = src[rmask], dst[rmask]
    r_coef = dinv_td[r_src] * dinv_td[r_dst]

    s_nodes = np.unique(np.concatenate([r_src, roots]))  # sorted
    s_graph = batch[s_nodes]

    # graph -> core: greedy balance of per-graph S edge weight, cap G_cap
    gw_td = np.bincount(s_graph, weights=deg_td[s_nodes], minlength=G)
    gw_bu = np.bincount(s_graph, weights=deg_bu[s_nodes], minlength=G)
    core_of_graph = np.empty(G, np.int64)
    glocal = np.empty(G, np.int64)
    counts = np.zeros(NCORES, np.int64)
    ld_td = np.zeros(NCORES)
    ld_bu = np.zeros(NCORES)
    for g in np.argsort(-(gw_td + gw_bu), kind="stable"):
        c = min((cc for cc in range(NCORES) if counts[cc] < G_cap),
                key=lambda cc: max(ld_td[cc] + gw_td[g], ld_bu[cc] + gw_bu[g]))
        core_of_graph[g] = c
        glocal[g] = counts[c]
        counts[c] += 1
        ld_td[c] += gw_td[g]
        ld_bu[c] += gw_bu[g]

    s_core = core_of_graph[s_graph]
    S_counts = np.bincount(s_core, minlength=NCORES)
    S_cap = max(_roundup(S_counts.max(), P), P)
    assert S_cap <= 512, f"S_cap={S_cap} > 512 unsupported"
    nSb = S_cap // P
    # assign S nodes to target-chunks (bins of P slots) balancing total edge
    # weight per bin so per-chunk k-tile counts are even across cores
    w_td_node = deg_td[s_nodes]
    w_bu_node = deg_bu[s_nodes]
    w_node = w_td_node + w_bu_node
    s_local = np.empty(len(s_nodes), np.int64)
    for c in range(NCORES):
        idx = np.flatnonzero(s_core == c)
        order = idx[np.argsort(-w_node[idx], kind="stable")]
        loads_td = np.zeros(nSb)
        loads_bu = np.zeros(nSb)
        fill = np.zeros(nSb, np.int64)
        for i in order:
            b = min((bb for bb in range(nSb) if fill[bb] < P),
                    key=lambda bb: max(loads_td[bb] + w_td_node[i],
                                       loads_bu[bb] + w_bu_node[i]))
            s_local[i] = b * P + fill[b]
            fill[b] += 1
            loads_td[b] += w_td_node[i]
            loads_bu[b] += w_bu_node[i]
    s_lookup = np.full(N, -1, np.int64)
    s_lookup[s_nodes] = s_local
    s_core_of_node = np.full(N, -1, np.int64)
    s_core_of_node[s_nodes] = s_core

    # layer-1 edge lists (targets in S, rows = neighbor node to gather)
    def _dir_edges(tgt_nodes, row_nodes, dinv):
        m = s_lookup[tgt_nodes] >= 0
        tgt = s_lookup[tgt_nodes[m]]
        rows = row_nodes[m]
        coef = dinv[row_nodes[m]] * dinv[tgt_nodes[m]]
        core = s_core_of_node[tgt_nodes[m]]
        return tgt, rows, coef.astype(np.float32), core

    td = _dir_edges(dst, src, dinv_td)   # l1_td aggregates at dst over src rows
    bu = _dir_edges(src, dst, dinv_bu)   # l1_bu aggregates at src over dst rows

    # per-(core, dir, target-chunk) k-tile counts must be uniform across cores
    # (SPMD: one program). Kc = global max tiles per chunk.
    nS = S_cap // P
    Kc = 1
    for tgt, rows, coef, core in (td, bu):
        for c in range(NCORES):
            sel = core == c
            ch = tgt[sel] // P
            for s in range(nS):
                n = int(np.count_nonzero(ch == s))
                Kc = max(Kc, -(-n // P))
    K = nS * Kc
    PACK = next(p for p in (5, 6, 8, 4, 3, 2, 1) if K % p == 0)
    E_cap = K * P

    # layer-2 aggregation matrix Pr[core, s_local, glocal]
    r_graph = batch[r_dst]
    assert np.all(core_of_graph[batch[r_src]] == core_of_graph[r_graph]), \
        "cross-core root edge unsupported"
    Pr = np.zeros((NCORES, S_cap, G_cap), np.float32)
    np.add.at(Pr, (core_of_graph[r_graph], s_lookup[r_src], glocal[r_graph]), r_coef)
    np.add.at(Pr, (core_of_graph[np.arange(G)], s_lookup[roots], glocal),
              dinv_td[roots] ** 2)

    in_maps = []
    for c in range(NCORES):
        m = {"pr": np.ascontiguousarray(Pr[c])}
        for name, (tgt, rows, coef, core) in (("td", td), ("bu", bu)):
            sel = core == c
            # chunk-relative target, laid out chunk s at k-tiles [s*Kc,(s+1)*Kc)
            tgt_p = np.zeros(E_cap, np.float32)
            coef_p = np.zeros(E_cap, np.float32)
            rows_p = np.zeros(E_cap, np.int64)
            tc, rc, cc = tgt[sel], rows[sel], coef[sel]
            ch = tc // P
            for s in range(nS):
                ss = ch == s
                n = int(np.count_nonzero(ss))
                o = s * Kc * P
                tgt_p[o:o + n] = (tc[ss] - s * P).astype(np.float32)
                coef_p[o:o + n] = cc[ss]
                rows_p[o:o + n] = rc[ss]
            xg = x[rows_p]                                    # [E_cap, F]
            # pack PACK k-tiles side by side: [K//PACK * P, PACK*F]
            xp4 = np.ascontiguousarray(
                xg.reshape(K // PACK, PACK, P, F).transpose(0, 2, 1, 3)
                  .reshape(K // PACK * P, PACK * F).astype(np.float16))
            m[f"xt_{name}"] = xp4
            m[f"tg_{name}"] = np.ascontiguousarray(tgt_p.reshape(K, P).T)
            m[f"cf_{name}"] = np.ascontiguousarray(coef_p.reshape(K, P).T)
        # root-feature + self-feature tables per S slot (pad rows zero)
        rfull = np.zeros((S_cap, F), np.float32)
        idx = np.flatnonzero(s_core == c)
        rfull[s_local[idx]] = x[roots[s_graph[idx]]]
        m["rfull"] = rfull
        xs = np.zeros((S_cap, F), np.float16)
        xs[s_local[idx]] = x[s_nodes[idx]].astype(np.float16)
        m["xs"] = xs
        for dn, dinv in (("td", dinv_td), ("bu", dinv_bu)):
            d2 = np.zeros((P, S_cap // P), np.float32)
            d2[s_local[idx] % P, s_local[idx] // P] = dinv[s_nodes[idx]] ** 2
            m[f"d2_{dn}"] = d2
        in_maps.append(m)

    meta = dict(F=F, S_cap=S_cap, K=K, G_cap=G_cap, counts=counts, G=G,
                Kc=Kc, PACK=PACK, core_of_graph=core_of_graph, glocal=glocal)
    return in_maps, meta


def _const_layout(F, H, C, S_cap, K, G_cap):
    """Column layout of the fused per-core constant matrix [P, W]."""
    nF, nS, nW2 = F // P, S_cap // P, (F + H) // P
    off = 0
    L = {}

    def add(name, w):
        nonlocal off
        L[name] = (off, w)
        off += w

    add("iota", P)
    for d in ("td", "bu"):
        add(f"tg_{d}", K)
        add(f"cf_{d}", K)
    L["__hot_end__"] = (off, 0)
    for d in ("td", "bu"):
        for f in range(nF):
            add(f"w1{d}{f}", H)
    for d in ("bu", "td"):
        for f in range(nW2):
            add(f"w2{d}{f}", H)
    for f in range(2 * H // P):
        add(f"wl{f}", C)
    add("d2_td", S_cap // P)
    add("d2_bu", S_cap // P)
    add("b1td", H)
    add("b1bu", H)
    add("b2bu", 1)
    add("b2td", 1)
    add("bl", C)
    add("ones", P)
    for s in range(nS):
        add(f"pr{s}", G_cap)
    for s in range(nS):
        add(f"rf{s}", F)
    return L, off


def _pack_consts(in_maps, inputs, meta, C):
    """Fold all per-core constants into one [P, W] matrix (single DMA)."""
    H = int(np.asarray(inputs["W_td1"]).shape[1])
    F, S_cap, K, G_cap = meta["F"], meta["S_cap"], meta["K"], meta["G_cap"]
    nF, nS, nW2 = F // P, S_cap // P, (F + H) // P
    L, W = _const_layout(F, H, C, S_cap, K, G_cap)
    g = lambda k: np.asarray(inputs[k], dtype=np.float32)

    base = np.zeros((P, W), np.float32)

    def put(name, block):
        o, w = L[name]
        base[:, o:o + w][tuple(slice(s) for s in block.shape)] = block

    put("iota", np.tile(np.arange(P, dtype=np.float32), (P, 1)))
    for d, wn in (("td", "W_td1"), ("bu", "W_bu1")):
        for f in range(nF):
            put(f"w1{d}{f}", g(wn)[f * P:(f + 1) * P, :])
    for d, wn in (("bu", "W_bu2"), ("td", "W_td2")):
        for f in range(nW2):
            put(f"w2{d}{f}", g(wn)[f * P:(f + 1) * P, :])
    for f in range(2 * H // P):
        put(f"wl{f}", g("W_lin")[f * P:(f + 1) * P, :])
    put("b1td", g("b_td1").reshape(1, H))
    put("b1bu", g("b_bu1").reshape(1, H))
    put("b2bu", g("b_bu2").reshape(H, 1))
    put("b2td", g("b_td2").reshape(H, 1))
    put("bl", g("b_lin").reshape(1, C))
    put("ones", np.ones((P, P), np.float32))

    for m in in_maps:
        cst = base.copy()
        o, w = L["iota"]  # noqa: F841
        for d in ("td", "bu"):
            cst[:, L[f"tg_{d}"][0]:L[f"tg_{d}"][0] + K] = m.pop(f"tg_{d}")
            cst[:, L[f"cf_{d}"][0]:L[f"cf_{d}"][0] + K] = m.pop(f"cf_{d}")
            o2_, w2_ = L[f"d2_{d}"]
            cst[:, o2_:o2_ + w2_] = m.pop(f"d2_{d}")
        pr = m.pop("pr")
        rf = m.pop("rfull")
        for s in range(nS):
            cst[:, L[f"pr{s}"][0]:L[f"pr{s}"][0] + G_cap] = pr[s * P:(s + 1) * P]
            cst[:, L[f"rf{s}"][0]:L[f"rf{s}"][0] + F] = rf[s * P:(s + 1) * P]
        m["cst"] = cst
    return H


# ----------------------------------------------------------------------------
# Device program
# ----------------------------------------------------------------------------

def _build_program(F, H, C, S_cap, K, G_cap, Kc, PACK, repeat=1):
    from contextlib import ExitStack

    import concourse.bacc as bacc
    from concourse.masks import make_identity
    import concourse.bass as bass  # noqa: F401
    import concourse.mybir as mybir
    import concourse.tile as tile

    dt = mybir.dt.float32
    dth = mybir.dt.float16
    E_cap = K * P
    nF = F // P
    nS = S_cap // P
    assert K == nS * Kc and K % PACK == 0
    nW2 = (F + H) // P
    CBW = F + 2 * H
    assert F % P == 0 and H == P and (F + H) % P == 0 and CBW % P == 0
    L, W = _const_layout(F, H, C, S_cap, K, G_cap)

    nc = bacc.Bacc("TRN2", target_bir_lowering=False, debug=False,
                   num_devices=NCORES)

    def din(name, shape):
        return nc.dram_tensor(name, list(shape), dt, kind="ExternalInput").ap()

    xt = {d: nc.dram_tensor(f"xt_{d}", [K // PACK * P, PACK * F], dth,
                            kind="ExternalInput").ap() for d in ("td", "bu")}
    xs_d = nc.dram_tensor("xs", [S_cap, F], dth, kind="ExternalInput").ap()
    cst_d = din("cst", [P, W])
    out = nc.dram_tensor("out", [G_cap, C], dt, kind="ExternalOutput").ap()

    eq, mul, sub = (mybir.AluOpType.is_equal, mybir.AluOpType.mult,
                    mybir.AluOpType.subtract)
    Relu, Exp, Ln = (mybir.ActivationFunctionType.Relu,
                     mybir.ActivationFunctionType.Exp,
                     mybir.ActivationFunctionType.Ln)

    with ExitStack() as ctx:
        tc = ctx.enter_context(tile.TileContext(nc))
        const = ctx.enter_context(tc.tile_pool(name="const",
                                                bufs=(1 if repeat == 1 else 2)))
        xpool = ctx.enter_context(tc.tile_pool(name="xp", bufs=8))
        ppool = ctx.enter_context(tc.tile_pool(name="pp", bufs=8))
        apool = ctx.enter_context(tc.tile_pool(name="ap", bufs=6))
        cpool = ctx.enter_context(tc.tile_pool(name="cp", bufs=max(nS, 2)))
        spool = ctx.enter_context(tc.tile_pool(name="sp", bufs=8))
        ps = ctx.enter_context(tc.tile_pool(name="ps", bufs=4, space="PSUM"))
        ps2 = ctx.enter_context(tc.tile_pool(name="ps2", bufs=3, space="PSUM"))

        ident = const.tile([P, P], dt, name="ident", tag="ident")
        make_identity(nc, ident[:])

        hot_w = L["__hot_end__"][0]
        for _rep in range(repeat):
            cst = const.tile([P, W], dt, name="cst", tag="cst")
            nc.sync.dma_start(cst[:, :hot_w], cst_d[:, :hot_w])
            nc.sync.dma_start(cst[:, hot_w:], cst_d[:, hot_w:])


            def C_(name, rows=None):
                o, w = L[name]
                if rows is None:
                    return cst[:, o:o + w]
                return cst[rows, o:o + w]

            # self-feature tiles (shared by both directions)
            xs_t = []
            for s in range(nS):
                t = xpool.tile([P, F], dth, tag="xs", name="xs")
                nc.sync.dma_start(t[:], xs_d[s * P:(s + 1) * P, :])
                xs_t.append(t)

            # stage 1: agg[d][s] [P(S-slots of chunk s), F]. The one-hot tile
            # is the stationary operand: one matmul per k-tile streaming all F
            # feature columns; chunk s accumulates over k in [s*Kc,(s+1)*Kc).
            agg_sb = {}
            for d in ("td", "bu"):
                agg_ps = [ps.tile([P, F], dt, tag="ps", name="aggps")
                          for _ in range(nS)]
                tgo, cfo = L[f"tg_{d}"][0], L[f"cf_{d}"][0]
                for kk in range(K // PACK):
                    xtile = xpool.tile([P, PACK * F], dth, tag="xt", name="xt")
                    nc.sync.dma_start(xtile[:], xt[d][kk * P:(kk + 1) * P, :])
                    for j in range(PACK):
                        k = kk * PACK + j
                        s = k // Kc
                        ptile = ppool.tile([P, P], dth, tag="pt", name="pt")
                        nc.vector.tensor_scalar(
                            out=ptile[:], in0=C_("iota"),
                            scalar1=cst[:, tgo + k:tgo + k + 1],
                            scalar2=cst[:, cfo + k:cfo + k + 1],
                            op0=eq, op1=mul)
                        nc.tensor.matmul(
                            out=agg_ps[s][:],
                            lhsT=ptile[:],
                            rhs=xtile[:, j * F:(j + 1) * F],
                            start=(k % Kc == 0), stop=(k % Kc == Kc - 1))
                # psum -> sbuf fused with the self-loop term d2*x[s]
                d2o = L[f"d2_{d}"][0]
                aggS = []
                for s in range(nS):
                    tmp = apool.tile([P, F], dt, tag="stmp", name="stmp")
                    nc.vector.tensor_scalar(
                        out=tmp[:], in0=xs_t[s][:],
                        scalar1=cst[:, d2o + s:d2o + s + 1], scalar2=None,
                        op0=mul)
                    t = apool.tile([P, F], dt, tag="aggS", name="aggS")
                    nc.vector.tensor_tensor(out=t[:], in0=agg_ps[s][:],
                                            in1=tmp[:],
                                            op=mybir.AluOpType.add)
                    aggS.append(t)
                agg_sb[d] = []
                for f in range(nF):
                    t = apool.tile([P, S_cap], dt, tag="agg", name="agg")
                    for s in range(nS):
                        tps = ps2.tile([P, P], dt, tag="ps2", name="tps")
                        nc.tensor.transpose(
                            out=tps[:], in_=aggS[s][:, f * P:(f + 1) * P],
                            identity=ident[:])
                        nc.scalar.copy(t[:, s * P:(s + 1) * P], tps[:])
                    agg_sb[d].append(t)

            # CBRBT tiles [P(S rows), CBW]: [relu(root) | relu(l1_bu) | relu(l1_td)]
            cbt = [cpool.tile([P, CBW], dt, tag="cbt", name="cbt") for _ in range(nS)]
            for s in range(nS):
                nc.vector.tensor_scalar(out=cbt[s][:, 0:F], in0=C_(f"rf{s}"),
                                        scalar1=0.0, scalar2=None,
                                        op0=mybir.AluOpType.max)

            # stage 2: l1 = aggT^T @ W1 + b1, relu -> cbt columns
            for di, d in enumerate(("bu", "td")):
                for s in range(nS):
                    h = ps2.tile([P, H], dt, tag="ps2", name="hps")
                    for f in range(nF):
                        nc.tensor.matmul(
                            out=h[:], lhsT=agg_sb[d][f][:, s * P:(s + 1) * P],
                            rhs=C_(f"w1{d}{f}"), start=(f == 0), stop=False)
                    nc.tensor.matmul(out=h[:], lhsT=C_("ones", rows=slice(0, 1)),
                                     rhs=C_(f"b1{d}", rows=slice(0, 1)),
                                     start=False, stop=True)
                    off = F + di * H
                    nc.vector.tensor_scalar(out=cbt[s][:, off:off + H],
                                            in0=h[:], scalar1=0.0,
                                            scalar2=None,
                                            op0=mybir.AluOpType.max)

            # stage 4: out2T[m] [P(col), G_cap] = cbt[:, mchunk]^T @ Pr
            o2_sb = []
            for m_ in range(CBW // P):
                o2 = ps2.tile([P, G_cap], dt, tag="ps2", name="o2ps")
                for s in range(nS):
                    nc.tensor.matmul(out=o2[:], lhsT=cbt[s][:, m_ * P:(m_ + 1) * P],
                                     rhs=C_(f"pr{s}"), start=(s == 0),
                                     stop=(s == nS - 1))
                t = spool.tile([P, G_cap], dt, tag="o2", name="o2sb")
                nc.scalar.copy(t[:], o2[:])
                o2_sb.append(t)

            # stage 5: pbT/ptT [H, G_cap] = relu(W2^T @ out2T_{R,d} + b2)
            tot = []
            nR = F // P
            for di, d in enumerate(("bu", "td")):
                tp = ps2.tile([P, G_cap], dt, tag="ps2", name="totps")
                for f in range(nW2):
                    rhs_t = o2_sb[f] if f < nR else o2_sb[nR + di]
                    nc.tensor.matmul(out=tp[:], lhsT=C_(f"w2{d}{f}"), rhs=rhs_t[:],
                                     start=(f == 0), stop=(f == nW2 - 1))
                t = spool.tile([P, G_cap], dt, tag=f"tot{di}", name=f"tot{di}")
                nc.vector.tensor_scalar(out=t[:], in0=tp[:],
                                        scalar1=C_(f"b2{d}"), scalar2=0.0,
                                        op0=mybir.AluOpType.add,
                                        op1=mybir.AluOpType.max)
                tot.append(t)

            # stage 6: logits [G_cap, C] = totalT^T @ W_lin + b_lin
            lg = ps2.tile([G_cap, C], dt, tag="ps2", name="lgps")
            for f in range(2 * H // P):
                nc.tensor.matmul(out=lg[:], lhsT=tot[f][:, :G_cap], rhs=C_(f"wl{f}"),
                                 start=(f == 0), stop=False)
            nc.tensor.matmul(out=lg[:], lhsT=C_("ones", rows=slice(0, 1))[:, :G_cap],
                             rhs=C_("bl", rows=slice(0, 1)), start=False, stop=True)

            # log_softmax rows (logits are O(1): no max-subtraction needed)
            ez = spool.tile([G_cap, C], dt, tag="ez", name="ez")
            se = spool.tile([G_cap, 1], dt, tag="se", name="se")
            nc.scalar.activation(ez[:], lg[:], Exp, accum_out=se[:])
            lse = spool.tile([G_cap, 1], dt, tag="lse", name="lse")
            nc.scalar.activation(lse[:], se[:], Ln)
            res = spool.tile([G_cap, C], dt, tag="res", name="res")
            nc.vector.tensor_scalar(out=res[:], in0=lg[:], scalar1=lse[:],
                                    scalar2=None, op0=sub)
            nc.sync.dma_start(out[:], res[:])

    nc.compile()
    return nc


_PROG_CACHE = {}


def _prepare_maps(inputs):
    in_maps, meta = _preprocess(inputs["x"], inputs["edge_index"],
                                inputs["batch"], inputs["num_graphs"])
    meta["C"] = int(np.asarray(inputs["W_lin"]).shape[1])
    meta["H"] = _pack_consts(in_maps, inputs, meta, meta["C"])
    return in_maps, meta


def _prepare(inputs):
    in_maps, meta = _prepare_maps(inputs)
    key = (meta["F"], meta["H"], meta["C"], meta["S_cap"], meta["K"],
           meta["G_cap"], meta["Kc"], meta["PACK"])
    if key not in _PROG_CACHE:
        _PROG_CACHE[key] = _build_program(*key)
    return _PROG_CACHE[key], in_maps, meta


def kernel(**inputs):
    from concourse.bass_utils import run_bass_kernel_spmd

    nc, in_maps, meta = _prepare(inputs)
    res = run_bass_kernel_spmd(nc, in_maps, list(range(NCORES)))
    G = meta["G"]
    cog, gl = meta["core_of_graph"], meta["glocal"]
    out = np.empty((G, meta["C"]), np.float32)
    for g in range(G):
        out[g] = res.results[cog[g]]["out"][gl[g]]
    return out



# revision 5
# speedup vs baseline: 1.1465x; 1.1465x over previous
"""BiGCN (nn_BiGCN_52716428591487) Trainium2 kernel.

Math: the model's output is log_softmax(cat(l2_bu[root], l2_td[root]) @ W_lin + b).
Only the layer-2 GCN values AT THE ROOT NODES matter, and GCNConv is linear in
its input features, so the whole network collapses to:

  agg1_d[v]  = sum_{e -> v} coef_d(e) * x[nbr(e)] + dinv_d[v]^2 * x[v]   (v in S)
  l1_d[v]    = agg1_d[v] @ W_d1 + b_d1
  cb/ct[v]   = relu([x[root(g(v))], l1_bu/td[v]])
  out2[g]    = sum_{s in S_g} Pr[s, g] * [relu(root), relu(l1_bu), relu(l1_td)][s]
  pb/pt[g]   = relu(out2_{R,bu/td}[g] @ W_2 + b_2)
  out[g]     = log_softmax([pb, pt][g] @ W_lin + b_lin)

where S = {sources of root-incident edges} + {roots} (~1.7k of 50k nodes) and
Pr is the (structure-only) layer-2 aggregation matrix.

Host does index-only preprocessing (degrees, edge selection, gather tables,
Pr); the device does every arithmetic op that touches x: the per-edge
coefficient scaling + aggregation (as one-hot x matmul on the PE), all four
GCN feature transforms, biases, relus, the linear head and log_softmax.

Sharding: graph-data parallel over 8 cores (graphs 0..12 -> core 0, ...).
Each core computes its graphs' rows of the output; the host concatenates.
"""

import numpy as np

P = 128
NCORES = 8


def _roundup(a, m):
    return -(-int(a) // m) * m


# ----------------------------------------------------------------------------
# Host preprocessing: index-only work + gather tables
# ----------------------------------------------------------------------------

def _preprocess(x, edge_index, batch, num_graphs):
    x = np.ascontiguousarray(np.asarray(x), dtype=np.float32)
    ei = np.asarray(edge_index)
    batch = np.asarray(batch).astype(np.int64)
    G = int(np.asarray(num_graphs))
    N, F = x.shape
    src = ei[0].astype(np.int64)
    dst = ei[1].astype(np.int64)

    assert np.all(np.diff(batch) >= 0), "batch must be sorted (contiguous graphs)"
    roots = np.searchsorted(batch, np.arange(G, dtype=np.int64))  # segment_min

    deg_td = 1.0 + np.bincount(dst, minlength=N).astype(np.float64)
    deg_bu = 1.0 + np.bincount(src, minlength=N).astype(np.float64)
    dinv_td = (1.0 / np.sqrt(deg_td)).astype(np.float32)
    dinv_bu = (1.0 / np.sqrt(deg_bu)).astype(np.float32)

    G_cap = max(-(-G // NCORES), 1)

    # S: sources of root-incident edges + roots
    is_root = np.zeros(N, bool)
    is_root[roots] = True
    rmask = is_root[dst]
    r_src, r_dst = src[rmask], dst[rmask]
    r_coef = dinv_td[r_src] * dinv_td[r_dst]

    s_nodes = np.unique(np.concatenate([r_src, roots]))  # sorted
    s_graph = batch[s_nodes]

    # graph -> core: greedy balance of per-graph S edge weight, cap G_cap
    gw_td = np.bincount(s_graph, weights=deg_td[s_nodes], minlength=G)
    gw_bu = np.bincount(s_graph, weights=deg_bu[s_nodes], minlength=G)
    core_of_graph = np.empty(G, np.int64)
    glocal = np.empty(G, np.int64)
    counts = np.zeros(NCORES, np.int64)
    ld_td = np.zeros(NCORES)
    ld_bu = np.zeros(NCORES)
    for g in np.argsort(-(gw_td + gw_bu), kind="stable"):
        c = min((cc for cc in range(NCORES) if counts[cc] < G_cap),
                key=lambda cc: max(ld_td[cc] + gw_td[g], ld_bu[cc] + gw_bu[g]))
        core_of_graph[g] = c
        glocal[g] = counts[c]
        counts[c] += 1
        ld_td[c] += gw_td[g]
        ld_bu[c] += gw_bu[g]

    s_core = core_of_graph[s_graph]
    S_counts = np.bincount(s_core, minlength=NCORES)
    S_cap = max(_roundup(S_counts.max(), P), P)
    assert S_cap <= 512, f"S_cap={S_cap} > 512 unsupported"
    nSb = S_cap // P
    # assign S nodes to target-chunks (bins of P slots) balancing total edge
    # weight per bin so per-chunk k-tile counts are even across cores
    w_td_node = deg_td[s_nodes]
    w_bu_node = deg_bu[s_nodes]
    w_node = w_td_node + w_bu_node
    s_local = np.empty(len(s_nodes), np.int64)
    for c in range(NCORES):
        idx = np.flatnonzero(s_core == c)
        order = idx[np.argsort(-w_node[idx], kind="stable")]
        loads_td = np.zeros(nSb)
        loads_bu = np.zeros(nSb)
        fill = np.zeros(nSb, np.int64)
        for i in order:
            b = min((bb for bb in range(nSb) if fill[bb] < P),
                    key=lambda bb: max(loads_td[bb] + w_td_node[i],
                                       loads_bu[bb] + w_bu_node[i]))
            s_local[i] = b * P + fill[b]
            fill[b] += 1
            loads_td[b] += w_td_node[i]
            loads_bu[b] += w_bu_node[i]
    s_lookup = np.full(N, -1, np.int64)
    s_lookup[s_nodes] = s_local
    s_core_of_node = np.full(N, -1, np.int64)
    s_core_of_node[s_nodes] = s_core

    # layer-1 edge lists (targets in S, rows = neighbor node to gather)
    def _dir_edges(tgt_nodes, row_nodes, dinv):
        m = s_lookup[tgt_nodes] >= 0
        tgt = s_lookup[tgt_nodes[m]]
        rows = row_nodes[m]
        coef = dinv[row_nodes[m]] * dinv[tgt_nodes[m]]
        core = s_core_of_node[tgt_nodes[m]]
        return tgt, rows, coef.astype(np.float32), core

    td = _dir_edges(dst, src, dinv_td)   # l1_td aggregates at dst over src rows
    bu = _dir_edges(src, dst, dinv_bu)   # l1_bu aggregates at src over dst rows

    # per-(core, dir, target-chunk) k-tile counts must be uniform across cores
    # (SPMD: one program). Kc = global max tiles per chunk.
    nS = S_cap // P
    Kc = 1
    for tgt, rows, coef, core in (td, bu):
        for c in range(NCORES):
            sel = core == c
            ch = tgt[sel] // P
            for s in range(nS):
                n = int(np.count_nonzero(ch == s))
                Kc = max(Kc, -(-n // P))
    K = nS * Kc
    PACK = next(p for p in (5, 6, 8, 4, 3, 2, 1) if K % p == 0)
    E_cap = K * P

    # layer-2 aggregation matrix Pr[core, s_local, glocal]
    r_graph = batch[r_dst]
    assert np.all(core_of_graph[batch[r_src]] == core_of_graph[r_graph]), \
        "cross-core root edge unsupported"
    Pr = np.zeros((NCORES, S_cap, G_cap), np.float32)
    np.add.at(Pr, (core_of_graph[r_graph], s_lookup[r_src], glocal[r_graph]), r_coef)
    np.add.at(Pr, (core_of_graph[np.arange(G)], s_lookup[roots], glocal),
              dinv_td[roots] ** 2)

    in_maps = []
    for c in range(NCORES):
        m = {"pr": np.ascontiguousarray(Pr[c])}
        for name, (tgt, rows, coef, core) in (("td", td), ("bu", bu)):
            sel = core == c
            # chunk-relative target, laid out chunk s at k-tiles [s*Kc,(s+1)*Kc)
            tgt_p = np.zeros(E_cap, np.float32)
            coef_p = np.zeros(E_cap, np.float32)
            rows_p = np.zeros(E_cap, np.int64)
            tc, rc, cc = tgt[sel], rows[sel], coef[sel]
            ch = tc // P
            for s in range(nS):
                ss = ch == s
                n = int(np.count_nonzero(ss))
                o = s * Kc * P
                tgt_p[o:o + n] = (tc[ss] - s * P).astype(np.float32)
                coef_p[o:o + n] = cc[ss]
                rows_p[o:o + n] = rc[ss]
            import ml_dtypes
            xg = x[rows_p]                                    # [E_cap, F]
            # pack PACK k-tiles side by side: [K//PACK * P, PACK*F]
            xp4 = np.ascontiguousarray(
                xg.reshape(K // PACK, PACK, P, F).transpose(0, 2, 1, 3)
                  .reshape(K // PACK * P, PACK * F).astype(ml_dtypes.float8_e4m3))
            m[f"xt_{name}"] = xp4
            m[f"tg_{name}"] = np.ascontiguousarray(tgt_p.reshape(K, P).T)
            m[f"cf_{name}"] = np.ascontiguousarray(coef_p.reshape(K, P).T)
        # root-feature + self-feature tables per S slot (pad rows zero)
        rfull = np.zeros((S_cap, F), np.float32)
        idx = np.flatnonzero(s_core == c)
        rfull[s_local[idx]] = x[roots[s_graph[idx]]]
        m["rfull"] = rfull
        xs = np.zeros((S_cap, F), np.float16)
        xs[s_local[idx]] = x[s_nodes[idx]].astype(np.float16)
        m["xs"] = xs
        for dn, dinv in (("td", dinv_td), ("bu", dinv_bu)):
            d2 = np.zeros((P, S_cap // P), np.float32)
            d2[s_local[idx] % P, s_local[idx] // P] = dinv[s_nodes[idx]] ** 2
            m[f"d2_{dn}"] = d2
        in_maps.append(m)

    meta = dict(F=F, S_cap=S_cap, K=K, G_cap=G_cap, counts=counts, G=G,
                Kc=Kc, PACK=PACK, core_of_graph=core_of_graph, glocal=glocal)
    return in_maps, meta


def _const_layout(F, H, C, S_cap, K, G_cap):
    """Column layout of the fused per-core constant matrix [P, W]."""
    nF, nS, nW2 = F // P, S_cap // P, (F + H) // P
    off = 0
    L = {}

    def add(name, w):
        nonlocal off
        L[name] = (off, w)
        off += w

    add("iota", P)
    for d in ("td", "bu"):
        add(f"tg_{d}", K)
        add(f"cf_{d}", K)
    L["__hot_end__"] = (off, 0)
    for d in ("td", "bu"):
        for f in range(nF):
            add(f"w1{d}{f}", H)
    for d in ("bu", "td"):
        for f in range(nW2):
            add(f"w2{d}{f}", H)
    for f in range(2 * H // P):
        add(f"wl{f}", C)
    add("d2_td", S_cap // P)
    add("d2_bu", S_cap // P)
    add("b1td", H)
    add("b1bu", H)
    add("b2bu", 1)
    add("b2td", 1)
    add("bl", C)
    add("ones", P)
    for s in range(nS):
        add(f"pr{s}", G_cap)
    for s in range(nS):
        add(f"rf{s}", F)
    return L, off


def _pack_consts(in_maps, inputs, meta, C):
    """Fold all per-core constants into one [P, W] matrix (single DMA)."""
    H = int(np.asarray(inputs["W_td1"]).shape[1])
    F, S_cap, K, G_cap = meta["F"], meta["S_cap"], meta["K"], meta["G_cap"]
    nF, nS, nW2 = F // P, S_cap // P, (F + H) // P
    L, W = _const_layout(F, H, C, S_cap, K, G_cap)
    g = lambda k: np.asarray(inputs[k], dtype=np.float32)

    base = np.zeros((P, W), np.float32)

    def put(name, block):
        o, w = L[name]
        base[:, o:o + w][tuple(slice(s) for s in block.shape)] = block

    put("iota", np.tile(np.arange(P, dtype=np.float32), (P, 1)))
    for d, wn in (("td", "W_td1"), ("bu", "W_bu1")):
        for f in range(nF):
            put(f"w1{d}{f}", g(wn)[f * P:(f + 1) * P, :])
    for d, wn in (("bu", "W_bu2"), ("td", "W_td2")):
        for f in range(nW2):
            put(f"w2{d}{f}", g(wn)[f * P:(f + 1) * P, :])
    for f in range(2 * H // P):
        put(f"wl{f}", g("W_lin")[f * P:(f + 1) * P, :])
    put("b1td", g("b_td1").reshape(1, H))
    put("b1bu", g("b_bu1").reshape(1, H))
    put("b2bu", g("b_bu2").reshape(H, 1))
    put("b2td", g("b_td2").reshape(H, 1))
    put("bl", g("b_lin").reshape(1, C))
    put("ones", np.ones((P, P), np.float32))

    for m in in_maps:
        cst = base.copy()
        o, w = L["iota"]  # noqa: F841
        for d in ("td", "bu"):
            cst[:, L[f"tg_{d}"][0]:L[f"tg_{d}"][0] + K] = m.pop(f"tg_{d}")
            cst[:, L[f"cf_{d}"][0]:L[f"cf_{d}"][0] + K] = m.pop(f"cf_{d}")
            o2_, w2_ = L[f"d2_{d}"]
            cst[:, o2_:o2_ + w2_] = m.pop(f"d2_{d}")
        pr = m.pop("pr")
        rf = m.pop("rfull")
        for s in range(nS):
            cst[:, L[f"pr{s}"][0]:L[f"pr{s}"][0] + G_cap] = pr[s * P:(s + 1) * P]
            cst[:, L[f"rf{s}"][0]:L[f"rf{s}"][0] + F] = rf[s * P:(s + 1) * P]
        m["cst"] = cst
    return H


# ----------------------------------------------------------------------------
# Device program
# ----------------------------------------------------------------------------

def _build_program(F, H, C, S_cap, K, G_cap, Kc, PACK, repeat=1):
    from contextlib import ExitStack

    import concourse.bacc as bacc
    from concourse.masks import make_identity
    import concourse.bass as bass  # noqa: F401
    import concourse.mybir as mybir
    import concourse.tile as tile

    dt = mybir.dt.float32
    dth = mybir.dt.float16
    dt8 = mybir.dt.float8e4
    E_cap = K * P
    nF = F // P
    nS = S_cap // P
    assert K == nS * Kc and K % PACK == 0
    nW2 = (F + H) // P
    CBW = F + 2 * H
    assert F % P == 0 and H == P and (F + H) % P == 0 and CBW % P == 0
    L, W = _const_layout(F, H, C, S_cap, K, G_cap)

    nc = bacc.Bacc("TRN2", target_bir_lowering=False, debug=False,
                   num_devices=NCORES)

    def din(name, shape):
        return nc.dram_tensor(name, list(shape), dt, kind="ExternalInput").ap()

    xt = {d: nc.dram_tensor(f"xt_{d}", [K // PACK * P, PACK * F], dt8,
                            kind="ExternalInput").ap() for d in ("td", "bu")}
    xs_d = nc.dram_tensor("xs", [S_cap, F], dth, kind="ExternalInput").ap()
    cst_d = din("cst", [P, W])
    out = nc.dram_tensor("out", [G_cap, C], dt, kind="ExternalOutput").ap()

    eq, mul, sub = (mybir.AluOpType.is_equal, mybir.AluOpType.mult,
                    mybir.AluOpType.subtract)
    Relu, Exp, Ln = (mybir.ActivationFunctionType.Relu,
                     mybir.ActivationFunctionType.Exp,
                     mybir.ActivationFunctionType.Ln)

    with ExitStack() as ctx:
        tc = ctx.enter_context(tile.TileContext(nc))
        const = ctx.enter_context(tc.tile_pool(name="const",
                                                bufs=(1 if repeat == 1 else 2)))
        xpool = ctx.enter_context(tc.tile_pool(name="xp", bufs=8))
        ppool = ctx.enter_context(tc.tile_pool(name="pp", bufs=8))
        apool = ctx.enter_context(tc.tile_pool(name="ap", bufs=6))
        cpool = ctx.enter_context(tc.tile_pool(name="cp", bufs=max(nS, 2)))
        spool = ctx.enter_context(tc.tile_pool(name="sp", bufs=8))
        ps = ctx.enter_context(tc.tile_pool(name="ps", bufs=4, space="PSUM"))
        ps2 = ctx.enter_context(tc.tile_pool(name="ps2", bufs=3, space="PSUM"))

        ident = const.tile([P, P], dt, name="ident", tag="ident")
        make_identity(nc, ident[:])

        hot_w = L["__hot_end__"][0]
        for _rep in range(repeat):
            cst = const.tile([P, W], dt, name="cst", tag="cst")
            nc.sync.dma_start(cst[:, :hot_w], cst_d[:, :hot_w])
            nc.sync.dma_start(cst[:, hot_w:], cst_d[:, hot_w:])


            def C_(name, rows=None):
                o, w = L[name]
                if rows is None:
                    return cst[:, o:o + w]
                return cst[rows, o:o + w]

            # self-feature tiles (shared by both directions)
            xs_t = []
            for s in range(nS):
                t = xpool.tile([P, F], dth, tag="xs", name="xs")
                nc.sync.dma_start(t[:], xs_d[s * P:(s + 1) * P, :])
                xs_t.append(t)

            # stage 1: agg[d][s] [P(S-slots of chunk s), F]. The one-hot tile
            # is the stationary operand: one matmul per k-tile streaming all F
            # feature columns; chunk s accumulates over k in [s*Kc,(s+1)*Kc).
            agg_sb = {}
            for d in ("td", "bu"):
                agg_ps = [ps.tile([P, F], dt, tag="ps", name="aggps")
                          for _ in range(nS)]
                tgo, cfo = L[f"tg_{d}"][0], L[f"cf_{d}"][0]
                for kk in range(K // PACK):
                    xtile = xpool.tile([P, PACK * F], dt8, tag="xt", name="xt")
                    nc.sync.dma_start(xtile[:], xt[d][kk * P:(kk + 1) * P, :])
                    for j in range(PACK):
                        k = kk * PACK + j
                        s = k // Kc
                        ptile = ppool.tile([P, P], dth, tag="pt", name="pt")
                        nc.vector.tensor_scalar(
                            out=ptile[:], in0=C_("iota"),
                            scalar1=cst[:, tgo + k:tgo + k + 1],
                            scalar2=cst[:, cfo + k:cfo + k + 1],
                            op0=eq, op1=mul)
                        nc.tensor.matmul(
                            out=agg_ps[s][:],
                            lhsT=ptile[:],
                            rhs=xtile[:, j * F:(j + 1) * F],
                            start=(k % Kc == 0), stop=(k % Kc == Kc - 1))
                # psum -> sbuf fused with the self-loop term d2*x[s]
                d2o = L[f"d2_{d}"][0]
                aggS = []
                for s in range(nS):
                    tmp = apool.tile([P, F], dt, tag="stmp", name="stmp")
                    nc.vector.tensor_scalar(
                        out=tmp[:], in0=xs_t[s][:],
                        scalar1=cst[:, d2o + s:d2o + s + 1], scalar2=None,
                        op0=mul)
                    t = apool.tile([P, F], dt, tag="aggS", name="aggS")
                    nc.vector.tensor_tensor(out=t[:], in0=agg_ps[s][:],
                                            in1=tmp[:],
                                            op=mybir.AluOpType.add)
                    aggS.append(t)
                agg_sb[d] = []
                for f in range(nF):
                    t = apool.tile([P, S_cap], dt, tag="agg", name="agg")
                    for s in range(nS):
                        tps = ps2.tile([P, P], dt, tag="ps2", name="tps")
                        nc.tensor.transpose(
                            out=tps[:], in_=aggS[s][:, f * P:(f + 1) * P],
                            identity=ident[:])
                        nc.scalar.copy(t[:, s * P:(s + 1) * P], tps[:])
                    agg_sb[d].append(t)

            # CBRBT tiles [P(S rows), CBW]: [relu(root) | relu(l1_bu) | relu(l1_td)]
            cbt = [cpool.tile([P, CBW], dt, tag="cbt", name="cbt") for _ in range(nS)]
            for s in range(nS):
                nc.vector.tensor_scalar(out=cbt[s][:, 0:F], in0=C_(f"rf{s}"),
                                        scalar1=0.0, scalar2=None,
                                        op0=mybir.AluOpType.max)

            # stage 2: l1 = aggT^T @ W1 + b1, relu -> cbt columns
            for di, d in enumerate(("bu", "td")):
                for s in range(nS):
                    h = ps2.tile([P, H], dt, tag="ps2", name="hps")
                    for f in range(nF):
                        nc.tensor.matmul(
                            out=h[:], lhsT=agg_sb[d][f][:, s * P:(s + 1) * P],
                            rhs=C_(f"w1{d}{f}"), start=(f == 0), stop=False)
                    nc.tensor.matmul(out=h[:], lhsT=C_("ones", rows=slice(0, 1)),
                                     rhs=C_(f"b1{d}", rows=slice(0, 1)),
                                     start=False, stop=True)
                    off = F + di * H
                    nc.vector.tensor_scalar(out=cbt[s][:, off:off + H],
                                            in0=h[:], scalar1=0.0,
                                            scalar2=None,
                                            op0=mybir.AluOpType.max)

            # stage 4: out2T[m] [P(col), G_cap] = cbt[:, mchunk]^T @ Pr
            o2_sb = []
            for m_ in range(CBW // P):
                o2 = ps2.tile([P, G_cap], dt, tag="ps2", name="o2ps")
                for s in range(nS):
                    nc.tensor.matmul(out=o2[:], lhsT=cbt[s][:, m_ * P:(m_ + 1) * P],
                                     rhs=C_(f"pr{s}"), start=(s == 0),
                                     stop=(s == nS - 1))
                t = spool.tile([P, G_cap], dt, tag="o2", name="o2sb")
                nc.scalar.copy(t[:], o2[:])
                o2_sb.append(t)

            # stage 5: pbT/ptT [H, G_cap] = relu(W2^T @ out2T_{R,d} + b2)
            tot = []
            nR = F // P
            for di, d in enumerate(("bu", "td")):
                tp = ps2.tile([P, G_cap], dt, tag="ps2", name="totps")
                for f in range(nW2):
                    rhs_t = o2_sb[f] if f < nR else o2_sb[nR + di]
                    nc.tensor.matmul(out=tp[:], lhsT=C_(f"w2{d}{f}"), rhs=rhs_t[:],
                                     start=(f == 0), stop=(f == nW2 - 1))
                t = spool.tile([P, G_cap], dt, tag=f"tot{di}", name=f"tot{di}")
                nc.vector.tensor_scalar(out=t[:], in0=tp[:],
                                        scalar1=C_(f"b2{d}"), scalar2=0.0,
                                        op0=mybir.AluOpType.add,
                                        op1=mybir.AluOpType.max)
                tot.append(t)

            # stage 6: logits [G_cap, C] = totalT^T @ W_lin + b_lin
            lg = ps2.tile([G_cap, C], dt, tag="ps2", name="lgps")
            for f in range(2 * H // P):
                nc.tensor.matmul(out=lg[:], lhsT=tot[f][:, :G_cap], rhs=C_(f"wl{f}"),
                                 start=(f == 0), stop=False)
            nc.tensor.matmul(out=lg[:], lhsT=C_("ones", rows=slice(0, 1))[:, :G_cap],
                             rhs=C_("bl", rows=slice(0, 1)), start=False, stop=True)

            # log_softmax rows (logits are O(1): no max-subtraction needed)
            ez = spool.tile([G_cap, C], dt, tag="ez", name="ez")
            se = spool.tile([G_cap, 1], dt, tag="se", name="se")
            nc.scalar.activation(ez[:], lg[:], Exp, accum_out=se[:])
            lse = spool.tile([G_cap, 1], dt, tag="lse", name="lse")
            nc.scalar.activation(lse[:], se[:], Ln)
            res = spool.tile([G_cap, C], dt, tag="res", name="res")
            nc.vector.tensor_scalar(out=res[:], in0=lg[:], scalar1=lse[:],
                                    scalar2=None, op0=sub)
            nc.sync.dma_start(out[:], res[:])

    nc.compile()
    return nc


_PROG_CACHE = {}


def _prepare_maps(inputs):
    in_maps, meta = _preprocess(inputs["x"], inputs["edge_index"],
                                inputs["batch"], inputs["num_graphs"])
    meta["C"] = int(np.asarray(inputs["W_lin"]).shape[1])
    meta["H"] = _pack_consts(in_maps, inputs, meta, meta["C"])
    return in_maps, meta


def _prepare(inputs):
    in_maps, meta = _prepare_maps(inputs)
    key = (meta["F"], meta["H"], meta["C"], meta["S_cap"], meta["K"],
           meta["G_cap"], meta["Kc"], meta["PACK"])
    if key not in _PROG_CACHE:
        _PROG_CACHE[key] = _build_program(*key)
    return _PROG_CACHE[key], in_maps, meta


def kernel(**inputs):
    from concourse.bass_utils import run_bass_kernel_spmd

    nc, in_maps, meta = _prepare(inputs)
    res = run_bass_kernel_spmd(nc, in_maps, list(range(NCORES)))
    G = meta["G"]
    cog, gl = meta["core_of_graph"], meta["glocal"]
    out = np.empty((G, meta["C"]), np.float32)
    for g in range(G):
        out[g] = res.results[cog[g]]["out"][gl[g]]
    return out

